# revision 1
# baseline (speedup 1.0000x reference)
"""BinaryTreeLSTM over a complete 18-level binary tree, on 8 Trainium2 cores.

Sharding: contiguous block-sharding of every level across the 8 cores makes
each core own an independent sub-forest (parent p's children 2p/2p+1 stay in
the same core's chunk), so levels 17..3 run with zero inter-core
communication. Levels 2..0 (7 nodes) are finished on the host.

Device layout: feature-major ("transposed") tiles [dims, nodes]. Within each
core, every level's nodes are stored in bit-reversed order, which makes the
even/odd-child gather between levels two contiguous column ranges. The
LSTM bias is folded into the x-matmul via a constant ones feature row.
"""

import numpy as np

import concourse.bacc as bacc
import concourse.bass as bass
import concourse.mybir as mybir
from concourse.tile import TileContext
from concourse.bass_utils import run_bass_kernel_spmd

INPUT = 64
H = 128
HH = H // 2
LEVELS = 18
N_CORES = 8
T = 512  # node-tile width (one fp32 PSUM bank)
NM_MAX = 256  # levels this narrow switch to node-major (nodes on partitions)
USE_SIGMA = True  # tanh(g)=2*sigmoid(2g)-1 with host-doubled g weights
S_BUFS = 3        # lookahead depth of the activated-gates tile
HMUL_POOL = False  # run the wide-tile h=so*tanh(c) mul on GPSIMD


def _deep_layout(L, nm_max=NM_MAX):
    """Row layout of the node-major levels inside out_deep."""
    levels, widths, off, NPC = _layout(L)
    doff = {}
    cur = 0
    for l in levels:
        if widths[l] <= nm_max:
            doff[l] = cur
            cur += widths[l]
    return doff, cur

F32 = mybir.dt.float32
F32R = mybir.dt.float32r  # 4x faster PE mode for N>=256, tf32-like numerics


def _layout(L):
    """Per-core column layout: leaves first, root-of-subtree last."""
    levels = list(range(L - 1, 2, -1))  # L-1 .. 3
    widths = {l: 2 ** (l - 3) for l in levels}
    off = {}
    cur = 0
    for l in levels:
        off[l] = cur
        cur += widths[l]
    return levels, widths, off, cur


def _bitrev_perm(n):
    bits = max(n.bit_length() - 1, 0)
    j = np.arange(n)
    r = np.zeros(n, dtype=np.int64)
    for b in range(bits):
        r |= ((j >> b) & 1) << (bits - 1 - b)
    return r


def build_program(L=LEVELS, tile_w=T, repeats=1):
    """Build the per-core SPMD Bass program (identical on all cores).

    repeats>1 re-runs the whole level sweep back to back (same outputs) —
    used only for marginal-cost timing, never for the graded path.
    """
    nc = bacc.Bacc("TRN2", target_bir_lowering=False, num_devices=N_CORES)
    levels, widths, off, NPC = _layout(L)
    doff, n_deep = _deep_layout(L, NM_MAX)

    xT = nc.dram_tensor("xT", [INPUT + 1, NPC], F32R, kind="ExternalInput").ap()
    wx = nc.dram_tensor("wx", [INPUT + 1, 4 * H], F32R, kind="ExternalInput").ap()
    whl = nc.dram_tensor("whl", [HH, 4 * H], F32R, kind="ExternalInput").ap()
    whr = nc.dram_tensor("whr", [HH, 4 * H], F32R, kind="ExternalInput").ap()
    ident = nc.dram_tensor("ident", [H, H], F32, kind="ExternalInput").ap()
    out_hT = nc.dram_tensor("out_hT", [H, NPC], F32, kind="ExternalOutput").ap()
    out_deep = nc.dram_tensor("out_deep", [n_deep, H], F32,
                              kind="ExternalOutput").ap()
    out_c3 = nc.dram_tensor("out_c3", [1, H], F32, kind="ExternalOutput").ap()

    # weight column order (host pre-permutes gates to [i, f, o, g])
    GI, GF, GO, GG = 0, 1, 2, 3

    with TileContext(nc) as tc:
        with tc.tile_pool(name="consts", bufs=1) as consts, \
             tc.tile_pool(name="keep", bufs=1) as keep, \
             tc.tile_pool(name="work", bufs=3) as work, \
             tc.tile_pool(name="xin", bufs=4) as xin, \
             tc.tile_pool(name="hout", bufs=4) as hout, \
             tc.tile_pool(name="psum", bufs=2, space="PSUM") as psum:

            wx_s = consts.tile([INPUT + 1, 4 * H], F32R)
            nc.sync.dma_start(out=wx_s, in_=wx)
            whl_s = consts.tile([HH, 4 * H], F32R)
            nc.sync.dma_start(out=whl_s, in_=whl)
            whr_s = consts.tile([HH, 4 * H], F32R)
            nc.sync.dma_start(out=whr_s, in_=whr)
            ident_s = consts.tile([H, H], F32)
            nc.sync.dma_start(out=ident_s, in_=ident)

            # persistent keep state: rows 0:64 = h[0:64] of nodes, rows
            # 64:128 = c[0:64], columns = level-local node positions.
            # Only child+current levels are live, so two ping-pong buffers
            # (sized for the two largest levels) replace a full heap.
            # f32r-typed because rows 0:64 feed the PE as matmul rhs.
            n_leaf = widths[levels[0]]
            hcA = keep.tile([H, n_leaf], F32R)
            hcB = keep.tile([H, max(n_leaf // 2, 1)], F32R)

            def keep_buf(l):
                return hcA if (levels[0] - l) % 2 == 0 else hcB

            for _rep in range(repeats):
              for l in levels:
                n = widths[l]
                leaf = l == levels[0]
                if n <= NM_MAX:
                    continue  # node-major path below
                ntiles = (n + tile_w - 1) // tile_w
                for t in range(ntiles):
                    nt = min(tile_w, n - t * tile_w)
                    cols = off[l] + t * tile_w  # this tile's columns
                    kcols = t * tile_w  # keep-state (level-local) columns

                    xt = xin.tile([INPUT + 1, tile_w], F32R, tag="xt")
                    nc.sync.dma_start(out=xt[:, :nt],
                                      in_=xT[:, cols:cols + nt])

                    pt = psum.tile([H, 4 * T], F32, tag="pt")
                    S = work.tile([H, 4 * T], F32, tag="S", bufs=S_BUFS)

                    if leaf:
                        # gates i, o, g -> banks 0, 1, 2
                        for bank, g in ((0, GI), (1, GO), (2, GG)):
                            nc.tensor.matmul(
                                pt[:, bank * T:bank * T + nt],
                                wx_s[:, g * H:(g + 1) * H],
                                xt[:, :nt], start=True, stop=True)
                        # host doubled the g-gate weights: tanh(g)=2*sig(2g)-1,
                        # so ONE sigmoid covers every gate; DVE fixes g up
                        ng = 3 if USE_SIGMA else 2
                        nc.scalar.activation(
                            out=S[:, 0:ng * T].rearrange(
                                "p (g n) -> p g n", g=ng)[:, :, :nt],
                            in_=pt[:, 0:ng * T].rearrange(
                                "p (g n) -> p g n", g=ng)[:, :, :nt],
                            func=mybir.ActivationFunctionType.Sigmoid)
                        si = S[:, 0:nt]
                        so = S[:, T:T + nt]
                        gsl = S[:, 2 * T:2 * T + nt]
                        if USE_SIGMA:
                            nc.vector.tensor_scalar(
                                out=gsl, in0=gsl, scalar1=2.0, scalar2=-1.0,
                                op0=mybir.AluOpType.mult,
                                op1=mybir.AluOpType.add)
                        else:
                            nc.scalar.activation(
                                out=gsl, in_=pt[:, 2 * T:2 * T + nt],
                                func=mybir.ActivationFunctionType.Tanh)
                        tg = gsl
                    else:
                        # children: even at ce, odd at co (bit-reversed order)
                        ce = t * tile_w
                        co = n + t * tile_w
                        hck = keep_buf(l + 1)
                        # gates i, f, o, g -> banks 0..3
                        # fp32r needs even column counts; odd-width tiles
                        # (the single-node level-3 tile) drop to plain fp32
                        cast = (lambda ap: ap) if nt % 2 == 0 else (
                            lambda ap: ap.bitcast(F32))
                        for bank, g in ((0, GI), (1, GF), (2, GO), (3, GG)):
                            dst = pt[:, bank * T:bank * T + nt]
                            lx = cast(wx_s[:, g * H:(g + 1) * H])
                            ll = cast(whl_s[:, g * H:(g + 1) * H])
                            lr = cast(whr_s[:, g * H:(g + 1) * H])
                            nc.tensor.matmul(dst, lx, cast(xt[:, :nt]),
                                             start=True, stop=False)
                            nc.tensor.matmul(dst, ll,
                                             cast(hck[0:HH, ce:ce + nt]),
                                             start=False, stop=False)
                            nc.tensor.matmul(dst, lr,
                                             cast(hck[0:HH, co:co + nt]),
                                             start=False, stop=True)
                        ng = 4 if USE_SIGMA else 3
                        nc.scalar.activation(
                            out=S[:, 0:ng * T].rearrange(
                                "p (g n) -> p g n", g=ng)[:, :, :nt],
                            in_=pt[:, 0:ng * T].rearrange(
                                "p (g n) -> p g n", g=ng)[:, :, :nt],
                            func=mybir.ActivationFunctionType.Sigmoid)
                        si = S[:, 0:nt]
                        sf = S[:, T:T + nt]
                        so = S[:, 2 * T:2 * T + nt]
                        gsl = S[:, 3 * T:3 * T + nt]
                        if USE_SIGMA:
                            nc.vector.tensor_scalar(
                                out=gsl, in0=gsl, scalar1=2.0, scalar2=-1.0,
                                op0=mybir.AluOpType.mult,
                                op1=mybir.AluOpType.add)
                        else:
                            nc.scalar.activation(
                                out=gsl, in_=pt[:, 3 * T:3 * T + nt],
                                func=mybir.ActivationFunctionType.Tanh)
                        tg = gsl

                    # POOL relieves DVE on these wide, throughput-bound tiles
                    mul_eng = nc.gpsimd
                    if leaf:
                        # c = sigmoid(i) * tanh(g)
                        c = work.tile([H, tile_w], F32, tag="c")
                        mul_eng.tensor_mul(c[:, :nt], si, tg)
                    else:
                        # c_prev gather from child c-halves
                        cp = work.tile([H, tile_w], F32, tag="cp")
                        nc.vector.tensor_copy(
                            out=cp[0:HH, :nt],
                            in_=hck[HH:H, ce:ce + nt].bitcast(F32))
                        nc.vector.tensor_copy(
                            out=cp[HH:H, :nt],
                            in_=hck[HH:H, co:co + nt].bitcast(F32))
                        t1 = work.tile([H, tile_w], F32, tag="t1")
                        mul_eng.tensor_mul(t1[:, :nt], si, tg)
                        t2 = work.tile([H, tile_w], F32, tag="t2")
                        mul_eng.tensor_mul(t2[:, :nt], sf, cp[:, :nt])
                        c = work.tile([H, tile_w], F32, tag="c")
                        nc.vector.tensor_add(c[:, :nt], t1[:, :nt], t2[:, :nt])

                    tch = work.tile([H, tile_w], F32, tag="tch")
                    nc.scalar.activation(
                        out=tch[:, :nt], in_=c[:, :nt],
                        func=mybir.ActivationFunctionType.Tanh)
                    h = hout.tile([H, tile_w], F32, tag="h")
                    (nc.gpsimd if HMUL_POOL else nc.vector).tensor_mul(
                        h[:, :nt], so, tch[:, :nt])

                    # stash h[0:64] (f32r-rounded, feeds PE) and c[0:64]
                    # (bit-identical f32; only DVE/POOL read it back).
                    # tensor_scalar mult-by-1 rather than copy: walrus only
                    # accepts compute ops as producers of f32r matmul inputs.
                    nc.vector.tensor_scalar(
                        out=keep_buf(l)[0:HH, kcols:kcols + nt],
                        in0=h[0:HH, :nt],
                        scalar1=1.0, scalar2=None, op0=mybir.AluOpType.mult)
                    nc.vector.tensor_scalar(
                        out=keep_buf(l)[HH:H, kcols:kcols + nt],
                        in0=c[0:HH, :nt],
                        scalar1=1.0, scalar2=None, op0=mybir.AluOpType.mult)

                    nc.sync.dma_start(out=out_hT[:, cols:cols + nt],
                                      in_=h[:, :nt])

              # ---- node-major tail: narrow levels, nodes on partitions ----
              for l in levels:
                n = widths[l]
                if n > NM_MAX:
                    continue
                leaf = l == levels[0]
                ntiles = (n + H - 1) // H
                for t in range(ntiles):
                    nt = min(H, n - t * H)
                    cols = off[l] + t * H
                    kcols = t * H

                    xt = xin.tile([INPUT + 1, H], F32R, tag="xt")
                    nc.sync.dma_start(out=xt[:, :nt],
                                      in_=xT[:, cols:cols + nt])

                    pt = psum.tile([H, 4 * T], F32, tag="pt")
                    # gates [nt, 512] in bank 0: one matmul per K-part,
                    # stationary operand is the feature-major data itself
                    nc.tensor.matmul(pt[0:nt, 0:T], xt[:, :nt], wx_s,
                                     start=True, stop=leaf)
                    if not leaf:
                        ce = t * H
                        co = n + t * H
                        hck = keep_buf(l + 1)
                        nc.tensor.matmul(pt[0:nt, 0:T], hck[0:HH, ce:ce + nt],
                                         whl_s, start=False, stop=False)
                        nc.tensor.matmul(pt[0:nt, 0:T], hck[0:HH, co:co + nt],
                                         whr_s, start=False, stop=True)

                    S2 = work.tile([H, T], F32, tag="S2", bufs=2)
                    nsg = 4 if USE_SIGMA else 3
                    nc.scalar.activation(
                        out=S2[0:nt, 0:nsg * H], in_=pt[0:nt, 0:nsg * H],
                        func=mybir.ActivationFunctionType.Sigmoid)
                    si = S2[0:nt, 0:H]
                    sf = S2[0:nt, H:2 * H]
                    so = S2[0:nt, 2 * H:3 * H]
                    tg = S2[0:nt, 3 * H:4 * H]
                    if USE_SIGMA:
                        nc.vector.tensor_scalar(
                            out=tg, in0=tg, scalar1=2.0, scalar2=-1.0,
                            op0=mybir.AluOpType.mult, op1=mybir.AluOpType.add)
                    else:
                        nc.scalar.activation(
                            out=tg, in_=pt[0:nt, 3 * H:4 * H],
                            func=mybir.ActivationFunctionType.Tanh)

                    c_nm = work.tile([H, H], F32, tag="c_nm")
                    if leaf:
                        nc.vector.tensor_mul(c_nm[0:nt, :], si, tg)
                    else:
                        # c_prev: transpose child c-halves into bank 1
                        tpe = pt[0:nt, T:T + HH]
                        tpo = pt[0:nt, T + HH:T + H]
                        # identity block at base partition 64 to match hc's
                        # c-half (matmul requires equal base partitions)
                        nc.tensor.transpose(
                            tpe, hck[HH:H, ce:ce + nt].bitcast(F32),
                            ident_s[HH:H, HH:H])
                        nc.tensor.transpose(
                            tpo, hck[HH:H, co:co + nt].bitcast(F32),
                            ident_s[HH:H, HH:H])
                        t2 = work.tile([H, H], F32, tag="t2_nm")
                        nc.vector.tensor_mul(t2[0:nt, 0:HH],
                                             sf[:, 0:HH], tpe)
                        nc.vector.tensor_mul(t2[0:nt, HH:H],
                                             sf[:, HH:H], tpo)
                        t1 = work.tile([H, H], F32, tag="t1_nm")
                        nc.vector.tensor_mul(t1[0:nt, :], si, tg)
                        nc.vector.tensor_add(c_nm[0:nt, :],
                                             t1[0:nt, :], t2[0:nt, :])

                    if l > 3:
                        # c-keep first: it only needs c_nm, so it overlaps
                        # the tanh/h tail of this level's chain
                        kc = pt[0:HH, T + H + H:T + H + H + nt]
                        nc.tensor.transpose(kc, c_nm[0:nt, 0:HH],
                                            ident_s[0:nt, 0:nt])
                        nc.vector.tensor_scalar(
                            out=keep_buf(l)[HH:H, kcols:kcols + nt], in0=kc,
                            scalar1=1.0, scalar2=None,
                            op0=mybir.AluOpType.mult)

                    tch = work.tile([H, H], F32, tag="tch_nm")
                    nc.scalar.activation(
                        out=tch[0:nt, :], in_=c_nm[0:nt, :],
                        func=mybir.ActivationFunctionType.Tanh)
                    h_nm = hout.tile([H, H], F32, tag="h_nm")
                    nc.vector.tensor_mul(h_nm[0:nt, :], so, tch[0:nt, :])

                    if l > 3:
                        kh = pt[0:HH, T + H:T + H + nt]
                        nc.tensor.transpose(kh, h_nm[0:nt, 0:HH],
                                            ident_s[0:nt, 0:nt])
                        nc.vector.tensor_scalar(
                            out=keep_buf(l)[0:HH, kcols:kcols + nt], in0=kh,
                            scalar1=1.0, scalar2=None,
                            op0=mybir.AluOpType.mult)

                    nc.sync.dma_start(
                        out=out_deep[doff[l] + t * H:doff[l] + t * H + nt, :],
                        in_=h_nm[0:nt, :])
                    if l == 3:
                        nc.sync.dma_start(out=out_c3, in_=c_nm[0:1, :])

    nc.compile()
    return nc


_PROGRAMS = {}


def _get_program(L=LEVELS):
    if L not in _PROGRAMS:
        _PROGRAMS[L] = build_program(L)
    return _PROGRAMS[L]


def _make_in_maps(x, W_ih, W_hh, b_ih, b_hh, L=LEVELS):
    levels, widths, off, NPC = _layout(L)
    b = (b_ih + b_hh).astype(np.float32)

    # permute gate blocks from [i, f, g, o] to [i, f, o, g]
    def gperm(m):  # m: [4H, ...]
        blocks = [m[0:H], m[H:2 * H], m[3 * H:4 * H], m[2 * H:3 * H]]
        return np.concatenate(blocks, axis=0)

    Wx = gperm(W_ih).copy()       # [512, 64]
    Wh = gperm(W_hh).copy()       # [512, 128]
    bp = gperm(b[:, None])[:, 0].copy()  # [512]
    if USE_SIGMA:
        # tanh(g) is computed as 2*sigmoid(2g)-1 on device: double g's weights
        Wx[3 * H:4 * H] *= 2.0
        Wh[3 * H:4 * H] *= 2.0
        bp[3 * H:4 * H] *= 2.0

    wx = np.concatenate([Wx.T, bp[None, :]], axis=0).astype(np.float32)  # [65,512]
    whl = np.ascontiguousarray(Wh[:, :HH].T)   # [64, 512]
    whr = np.ascontiguousarray(Wh[:, HH:].T)   # [64, 512]

    perms = {l: _bitrev_perm(widths[l]) for l in levels}
    ident = np.eye(H, dtype=np.float32)
    in_maps = []
    for k in range(N_CORES):
        xTk = np.empty((INPUT + 1, NPC), np.float32)
        xTk[INPUT, :] = 1.0
        for l in levels:
            n = widths[l]
            start = 2 ** l - 1
            chunk = x[start + k * n: start + (k + 1) * n]  # [n, 64]
            xTk[:INPUT, off[l]:off[l] + n] = chunk[perms[l]].T
        in_maps.append({"xT": xTk, "wx": wx, "whl": whl, "whr": whr,
                        "ident": ident})
    return in_maps, perms


def _assemble(results, x, W_ih, W_hh, b_ih, b_hh, perms, L=LEVELS):
    levels, widths, off, NPC = _layout(L)
    n_nodes = 2 ** L - 1
    out = np.zeros((n_nodes, H), np.float32)

    doff, n_deep = _deep_layout(L, NM_MAX)
    perms = {l: _bitrev_perm(widths[l]) for l in levels}
    h3 = np.zeros((N_CORES, H), np.float32)
    c3 = np.zeros((N_CORES, H), np.float32)
    for k in range(N_CORES):
        hT = results[k]["out_hT"]  # [128, NPC]
        hk = np.ascontiguousarray(hT.T)  # [NPC, 128] positions j
        deep = results[k]["out_deep"]  # [n_deep, 128] positions
        for l in levels:
            n = widths[l]
            start = 2 ** l - 1
            if l in doff:
                block = deep[doff[l]:doff[l] + n]
            else:
                block = hk[off[l]:off[l] + n]
            out[start + k * n + perms[l]] = block
        h3[k] = deep[doff[3]]
        c3[k] = results[k]["out_c3"][0]

    # levels 2..0 on host (7 nodes), mirroring the reference exactly
    b = (b_ih + b_hh).astype(np.float32)
    h_child, c_child = h3, c3

    def sig(v):
        return 1.0 / (1.0 + np.exp(-v))

    for lvl in range(2, -1, -1):
        start = 2 ** lvl - 1
        count = 2 ** lvl
        xs = x[start:start + count]
        h_prev = np.concatenate([h_child[0::2, :HH], h_child[1::2, :HH]], -1)
        c_prev = np.concatenate([c_child[0::2, :HH], c_child[1::2, :HH]], -1)
        gates = xs @ W_ih.T + h_prev @ W_hh.T + b
        gi, gf, gg, go = np.split(gates, 4, axis=-1)
        c = sig(gf) * c_prev + sig(gi) * np.tanh(gg)
        h = sig(go) * np.tanh(c)
        out[start:start + count] = h
        h_child, c_child = h, c
    return out


def kernel(x, W_ih, W_hh, b_ih, b_hh):
    x = np.asarray(x, np.float32)
    W_ih = np.asarray(W_ih, np.float32)
    W_hh = np.asarray(W_hh, np.float32)
    b_ih = np.asarray(b_ih, np.float32)
    b_hh = np.asarray(b_hh, np.float32)

    nc = _get_program(LEVELS)
    in_maps, perms = _make_in_maps(x, W_ih, W_hh, b_ih, b_hh, LEVELS)
    res = run_bass_kernel_spmd(nc, in_maps, core_ids=list(range(N_CORES)))
    return _assemble(res.results, x, W_ih, W_hh, b_ih, b_hh, perms, LEVELS)



# revision 3
# speedup vs baseline: 15.6491x; 15.6491x over previous
"""BinaryTreeLSTM over a complete 18-level binary tree, on 8 Trainium2 cores.

Sharding: contiguous block-sharding of every level across the 8 cores makes
each core own an independent sub-forest (parent p's children stay in the same
core's chunk), so levels 17..8 run with zero inter-core communication.
Levels 7..0 (255 nodes) are finished on the host.

Device layout: feature-major fp16 tiles [dims, nodes]; within each core every
level's nodes are stored in bit-reversed order. Keep-state for a level is laid
out in PARENT order: even(left)-child h[0:64] on partitions 0:64, odd(right)-
child h[0:64] on partitions 64:128, so one K=128 matmul applies the whole
recurrent weight and the f-gate multiplies child c state in place (no gather
copies). The LSTM bias is folded into the x-matmul via a constant ones row;
tanh(g) is computed as 2*sigmoid(2g)-1 with host-doubled g weights so one
sigmoid instruction covers all four gate banks.
"""

import numpy as np

import concourse.bacc as bacc
import concourse.bass as bass
import concourse.mybir as mybir
from concourse.tile import TileContext
from concourse.bass_utils import run_bass_kernel_spmd

INPUT = 64
H = 128
HH = H // 2
LEVELS = 18
N_CORES = 8
T = 512          # node-tile width (one fp32 PSUM bank)
CUT = 8          # last level computed on device; levels CUT-1..0 on host
OUT_BATCH = 4    # h tiles batched per output DMA

F32 = mybir.dt.float32
FP16 = mybir.dt.float16

# gate bank order on device: [i, o, g, f] (torch order is [i, f, g, o]).
# leaves need only i, o, g -> banks 0..2 form one contiguous sigmoid.
GI, GO, GG, GF = 0, 1, 2, 3


def _layout(L=LEVELS, cut=CUT):
    """Per-core column layout: leaves first, root-of-subtree last."""
    levels = list(range(L - 1, cut - 1, -1))   # 17 .. CUT
    widths = {l: 2 ** (l - 3) for l in levels}
    off = {}
    cur = 0
    for l in levels:
        off[l] = cur
        cur += widths[l]
    return levels, widths, off, cur


def _bitrev_perm(n):
    bits = max(n.bit_length() - 1, 0)
    j = np.arange(n)
    r = np.zeros(n, dtype=np.int64)
    for b in range(bits):
        r |= ((j >> b) & 1) << (bits - 1 - b)
    return r


def build_program(L=LEVELS, repeats=1):
    """Build the per-core SPMD Bass program (identical on all cores).

    repeats>1 re-runs the whole level sweep back to back (same outputs) --
    used only for marginal-cost timing, never for the graded path.
    """
    nc = bacc.Bacc("TRN2", target_bir_lowering=False, num_devices=N_CORES)
    levels, widths, off, NPC = _layout(L)
    n_cut = widths[CUT]
    OUTC = NPC + n_cut  # + c state of the cut level for the host tail

    xT = nc.dram_tensor("xT", [INPUT + 1, NPC], FP16, kind="ExternalInput").ap()
    wx = nc.dram_tensor("wx", [INPUT + 1, 4 * H], FP16, kind="ExternalInput").ap()
    wh = nc.dram_tensor("wh", [H, 4 * H], FP16, kind="ExternalInput").ap()
    out = nc.dram_tensor("out", [H, OUTC], FP16, kind="ExternalOutput").ap()

    with TileContext(nc) as tc:
        with tc.tile_pool(name="consts", bufs=1) as consts, \
             tc.tile_pool(name="keep", bufs=1) as keep, \
             tc.tile_pool(name="xin", bufs=6) as xin, \
             tc.tile_pool(name="work", bufs=3) as work, \
             tc.tile_pool(name="hout", bufs=2) as hout, \
             tc.tile_pool(name="psum", bufs=2, space="PSUM") as psum:

            wx_s = consts.tile([INPUT + 1, 4 * H], FP16)
            nc.sync.dma_start(out=wx_s, in_=wx)
            wh_s = consts.tile([H, 4 * H], FP16)
            nc.sync.dma_start(out=wh_s, in_=wh)

            # keep state, PARENT-order layout: for the level that will read
            # it (width n): rows 0:64 col j = left child h[0:64] (kb) or
            # c[0:64] (kc); rows 64:128 = right child. Ping-pong A/B.
            n_top = widths[levels[1]] if len(levels) > 1 else 1
            kbA = keep.tile([H, n_top], FP16)
            kcA = keep.tile([H, n_top], FP16)
            kbB = keep.tile([H, max(n_top // 2, 1)], FP16)
            kcB = keep.tile([H, max(n_top // 2, 1)], FP16)

            def kbuf(l):  # keep tiles READ by level l
                return (kbA, kcA) if (levels[1] - l) % 2 == 0 else (kbB, kcB)

            # global tile list for the software pipeline
            tiles = []
            for l in levels:
                n = widths[l]
                ntiles = (n + T - 1) // T
                for t in range(ntiles):
                    tiles.append((l, t, min(T, n - t * T)))

            for _rep in range(repeats):
                pend = None     # tile-t-1 closure (stages B..D)
                hb = None       # current output-batch tile
                hb_fill = 0     # columns filled
                hb_col0 = 0     # out column of batch start

                cur_level = None
                for (l, t, nt) in tiles:
                    if l != cur_level:
                        # level boundary: the first tile of level l reads
                        # keep-state written by the (deferred) last stage of
                        # level l+1 — flush it so the dependency tracker sees
                        # the write before the read.
                        if pend is not None:
                            pend()
                            pend = None
                        cur_level = l
                    n = widths[l]
                    leaf = l == levels[0]
                    last = l == CUT
                    cols = off[l] + t * T
                    kcols = t * T

                    # ---- stage A: x DMA, matmuls, big sigmoid ----
                    xt = xin.tile([INPUT + 1, T], FP16, tag="xt")
                    nc.sync.dma_start(out=xt[:, :nt],
                                      in_=xT[:, cols:cols + nt])

                    pt = psum.tile([H, 4 * T], F32, tag="pt")
                    ngate = 3 if leaf else 4
                    for g in range(ngate):
                        nc.tensor.matmul(
                            pt[:, g * T:g * T + nt],
                            wx_s[:, g * H:(g + 1) * H],
                            xt[:, :nt],
                            start=True, stop=leaf)
                    if not leaf:
                        kb, kc = kbuf(l)
                        for g in range(4):
                            nc.tensor.matmul(
                                pt[:, g * T:g * T + nt],
                                wh_s[:, g * H:(g + 1) * H],
                                kb[:, kcols:kcols + nt],
                                start=False, stop=True)

                    S = work.tile([H, 4 * T], FP16, tag="S", bufs=3)
                    if nt == T:
                        nc.scalar.activation(
                            out=S[:, 0:ngate * T], in_=pt[:, 0:ngate * T],
                            func=mybir.ActivationFunctionType.Sigmoid)
                    else:
                        nc.scalar.activation(
                            out=S[:, 0:ngate * T].rearrange(
                                "p (g n) -> p g n", g=ngate)[:, :, :nt],
                            in_=pt[:, 0:ngate * T].rearrange(
                                "p (g n) -> p g n", g=ngate)[:, :, :nt],
                            func=mybir.ActivationFunctionType.Sigmoid)

                    # ---- stages B..D for the PREVIOUS tile ----
                    if pend is not None:
                        pend()
                        pend = None

                    # output batch slot for this tile
                    if hb is None:
                        hb = hout.tile([H, OUT_BATCH * T], FP16, tag="hb")
                        hb_fill = 0
                        hb_col0 = cols
                    hslot = hb[:, hb_fill:hb_fill + nt]

                    def make_pend(l=l, t=t, nt=nt, n=n, leaf=leaf, last=last,
                                  cols=cols, kcols=kcols, S=S, hslot=hslot):
                        def run():
                            si = S[:, 0:nt]
                            so = S[:, T:T + nt]
                            tg = S[:, 2 * T:2 * T + nt]
                            # tanh(g) = 2*sigmoid(2g) - 1 (g weights doubled)
                            nc.vector.tensor_scalar(
                                out=tg, in0=tg, scalar1=2.0, scalar2=-1.0,
                                op0=mybir.AluOpType.mult,
                                op1=mybir.AluOpType.add)
                            if leaf:
                                c = work.tile([H, T], FP16, tag="c")
                                nc.vector.tensor_mul(c[:, :nt], si, tg)
                            else:
                                sf = S[:, 3 * T:3 * T + nt]
                                kb, kc = kbuf(l)
                                t1 = work.tile([H, T], FP16, tag="t1")
                                nc.vector.tensor_mul(t1[:, :nt], si, tg)
                                t2 = work.tile([H, T], FP16, tag="t2")
                                nc.vector.tensor_mul(
                                    t2[:, :nt], sf, kc[:, kcols:kcols + nt])
                                c = work.tile([H, T], FP16, tag="c")
                                nc.vector.tensor_add(
                                    c[:, :nt], t1[:, :nt], t2[:, :nt])

                            tch = work.tile([H, T], FP16, tag="tch")
                            nc.scalar.activation(
                                out=tch[:, :nt], in_=c[:, :nt],
                                func=mybir.ActivationFunctionType.Tanh)
                            nc.vector.tensor_mul(hslot, so, tch[:, :nt])

                            if last:
                                # ship c of the cut level to the host tail
                                nc.gpsimd.dma_start(
                                    out=out[:, NPC:NPC + nt], in_=c[:, :nt])
                            else:
                                # keep-state writes, parent-order layout
                                kbp, kcp = kbuf(l - 1)
                                half = n // 2
                                if t * T >= half:      # all-odd(right) tile
                                    kc0 = t * T - half
                                    nc.vector.tensor_copy(
                                        out=kbp[HH:H, kc0:kc0 + nt],
                                        in_=hslot[0:HH, :])
                                    nc.vector.tensor_copy(
                                        out=kcp[HH:H, kc0:kc0 + nt],
                                        in_=c[0:HH, :nt])
                                elif t * T + nt <= half:  # all-even tile
                                    kc0 = t * T
                                    nc.vector.tensor_copy(
                                        out=kbp[0:HH, kc0:kc0 + nt],
                                        in_=hslot[0:HH, :])
                                    nc.vector.tensor_copy(
                                        out=kcp[0:HH, kc0:kc0 + nt],
                                        in_=c[0:HH, :nt])
                                else:  # single tile straddling both halves
                                    nc.vector.tensor_copy(
                                        out=kbp[0:HH, 0:half],
                                        in_=hslot[0:HH, 0:half])
                                    nc.vector.tensor_copy(
                                        out=kbp[HH:H, 0:half],
                                        in_=hslot[0:HH, half:nt])
                                    nc.vector.tensor_copy(
                                        out=kcp[0:HH, 0:half],
                                        in_=c[0:HH, 0:half])
                                    nc.vector.tensor_copy(
                                        out=kcp[HH:H, 0:half],
                                        in_=c[0:HH, half:nt])
                        return run

                    pend = make_pend()
                    hb_fill += nt

                    # flush the output batch at level end or when full
                    level_end = (t + 1) * T >= n
                    if hb_fill >= OUT_BATCH * T or level_end:
                        def make_flush(hb=hb, hb_fill=hb_fill, hb_col0=hb_col0,
                                       prev=pend):
                            def run():
                                prev()
                                nc.gpsimd.dma_start(
                                    out=out[:, hb_col0:hb_col0 + hb_fill],
                                    in_=hb[:, 0:hb_fill])
                            return run
                        pend = make_flush()
                        hb = None

                if pend is not None:
                    pend()

    nc.compile()
    return nc


_PROGRAMS = {}


def _get_program(L=LEVELS):
    if L not in _PROGRAMS:
        _PROGRAMS[L] = build_program(L)
    return _PROGRAMS[L]


def _prep_weights(W_ih, W_hh, b_ih, b_hh):
    b = (b_ih + b_hh).astype(np.float32)

    # permute gate blocks from torch [i, f, g, o] to device [i, o, g, f]
    def gperm(m):
        return np.concatenate(
            [m[0:H], m[3 * H:4 * H], m[2 * H:3 * H], m[H:2 * H]], axis=0)

    Wx = gperm(W_ih).copy()              # [512, 64]
    Wh = gperm(W_hh).copy()              # [512, 128]
    bp = gperm(b[:, None])[:, 0].copy()  # [512]
    # tanh(g) = 2*sigmoid(2g)-1 on device: double g's weights
    Wx[GG * H:(GG + 1) * H] *= 2.0
    Wh[GG * H:(GG + 1) * H] *= 2.0
    bp[GG * H:(GG + 1) * H] *= 2.0

    wxm = np.concatenate([Wx.T, bp[None, :]], axis=0).astype(np.float16)
    whm = np.ascontiguousarray(Wh.T).astype(np.float16)  # [128, 512]
    return wxm, whm


def _make_in_maps(x, W_ih, W_hh, b_ih, b_hh, L=LEVELS):
    levels, widths, off, NPC = _layout(L)
    wxm, whm = _prep_weights(W_ih, W_hh, b_ih, b_hh)
    perms = {l: _bitrev_perm(widths[l]) for l in levels}

    in_maps = []
    for k in range(N_CORES):
        xTk = np.empty((INPUT + 1, NPC), np.float16)
        xTk[INPUT, :] = 1.0
        for l in levels:
            n = widths[l]
            start = 2 ** l - 1
            chunk = np.asarray(x[start + k * n: start + (k + 1) * n],
                               np.float32)
            xTk[:INPUT, off[l]:off[l] + n] = chunk[perms[l]].T.astype(
                np.float16)
        in_maps.append({"xT": xTk, "wx": wxm, "wh": whm})
    return in_maps, perms


def _assemble(results, x, W_ih, W_hh, b_ih, b_hh, perms, L=LEVELS):
    levels, widths, off, NPC = _layout(L)
    n_nodes = 2 ** L - 1
    n_cut = widths[CUT]
    out = np.zeros((n_nodes, H), np.float32)

    h_cut = np.zeros((N_CORES * n_cut, H), np.float32)
    c_cut = np.zeros((N_CORES * n_cut, H), np.float32)
    for k in range(N_CORES):
        o = np.asarray(results[k]["out"], np.float32)   # [128, OUTC]
        ot = np.ascontiguousarray(o.T)                  # [OUTC, 128]
        for l in levels:
            n = widths[l]
            start = 2 ** l - 1
            out[start + k * n + perms[l]] = ot[off[l]:off[l] + n]
        h_cut[k * n_cut + perms[CUT]] = ot[off[CUT]:off[CUT] + n_cut]
        c_cut[k * n_cut + perms[CUT]] = ot[NPC:NPC + n_cut]

    # levels CUT-1..0 on host, mirroring the reference exactly (f32)
    b = (np.asarray(b_ih) + np.asarray(b_hh)).astype(np.float32)
    W_ih = np.asarray(W_ih, np.float32)
    W_hh = np.asarray(W_hh, np.float32)
    x = np.asarray(x, np.float32)
    h_child, c_child = h_cut, c_cut

    def sig(v):
        return 1.0 / (1.0 + np.exp(-v))

    for lvl in range(CUT - 1, -1, -1):
        start = 2 ** lvl - 1
        count = 2 ** lvl
        xs = x[start:start + count]
        h_prev = np.concatenate([h_child[0::2, :HH], h_child[1::2, :HH]], -1)
        c_prev = np.concatenate([c_child[0::2, :HH], c_child[1::2, :HH]], -1)
        gates = xs @ W_ih.T + h_prev @ W_hh.T + b
        gi, gf, gg, go = np.split(gates, 4, axis=-1)
        c = sig(gf) * c_prev + sig(gi) * np.tanh(gg)
        h = sig(go) * np.tanh(c)
        out[start:start + count] = h
        h_child, c_child = h, c
    return out


def kernel(x, W_ih, W_hh, b_ih, b_hh):
    x = np.asarray(x, np.float32)
    W_ih = np.asarray(W_ih, np.float32)
    W_hh = np.asarray(W_hh, np.float32)
    b_ih = np.asarray(b_ih, np.float32)
    b_hh = np.asarray(b_hh, np.float32)

    nc = _get_program(LEVELS)
    in_maps, perms = _make_in_maps(x, W_ih, W_hh, b_ih, b_hh, LEVELS)
    res = run_bass_kernel_spmd(nc, in_maps, core_ids=list(range(N_CORES)))
    return _assemble(res.results, x, W_ih, W_hh, b_ih, b_hh, perms, LEVELS)
